# revision 2
# baseline (speedup 1.0000x reference)
"""Trainium2 Bass kernel for nn_LongTermAttention (continuous-basis long-term attention).

Strategy: pure data-parallel over batch (B=8 -> one batch element per NeuronCore).
Per core, the computation is restructured so the full [q, NB] score matrix is
never materialized:

  maskT[l, h]  = sigmoid(W_mask.T(stationary) contracted with k) + b_mask      (PE + ACT)
  kmT[l, h]    = k * maskT                                                     (DVE)
  BmatT[j, n~] = kmT.T @ Gs_perm   (n~ = sigma-deinterleaved basis order)      (PE)
  v_mu/v_sig   = kmT.T @ (Gs @ w_mu / w_sigma)   (host-folded into Gs_aug)     (PE)
  vals[n~, j]  = BmatT.T @ Wv.T                                                (PE)
  u            = [v_mu; v_sig] @ Wk.T / sqrt(d)                                (PE)
  Wtilde       = per-head block-diag expansion of u, contracted with Wq        (PE)
  mu_pre[32,q] = Wtilde.T @ qT  (rows 0-15: mu channel, 16-31: sigma channel)  (PE)
  grids        = sigmoid/softplus/recip/ln -> per-(s,h,q) quadratic coeffs     (ACT/DVE)
  g[n~, q]     = K=3 matmul: [lin^2; lin; 1].T @ [q1; q2; q3]  (the Gaussian
                 exponent incl. normalization), r = Exp(g) on PSUM eviction    (PE + ACT)
  ctx.T[d, q]  = vals_h.T @ r_h  (2 heads per PSUM tile via column tiling)     (PE)
  out[q, o]    = ctx.T.T @ Wo.T                                                (PE)

All matmuls run as float32r (fast fp32 path). Weights are pre-transposed and
basis-derived constants are precomputed on the host as part of input layout.
"""
import os
from contextlib import ExitStack

import numpy as np

import concourse.bass as bass
import concourse.tile as tile
from concourse import bacc, mybir
from concourse.bass_utils import run_bass_kernel_spmd
from concourse.masks import make_identity

F32 = mybir.dt.float32
F32R = mybir.dt.float32r
AF = mybir.ActivationFunctionType
AL = mybir.AluOpType

L = 2048          # memory length
NB = 512          # num basis
NB2 = 256         # per-sigma basis count
HID = 1024
H = 16
D = 64
B = 8
Q = 2048
LT = L // 128     # 16
JT = HID // 128   # 8
QTI = Q // 128    # 16
SIGMAS = (0.005, 0.01)
TWO_PI = 6.283185307179586


def build_nc():
    nc = bacc.Bacc("TRN2", target_bir_lowering=False, debug=False)

    k_d = nc.dram_tensor("k", [L, HID], F32, kind="ExternalInput").ap()
    qt_d = nc.dram_tensor("qt", [HID, Q], F32, kind="ExternalInput").ap()
    wm_d = nc.dram_tensor("wmT", [L, L], F32, kind="ExternalInput").ap()
    gs_d = nc.dram_tensor("gs_aug", [L, NB + 2], F32, kind="ExternalInput").ap()
    wv_d = nc.dram_tensor("wvT", [HID, HID], F32, kind="ExternalInput").ap()
    wk_d = nc.dram_tensor("wkT", [HID, HID], F32, kind="ExternalInput").ap()
    wq_d = nc.dram_tensor("wq", [HID, HID], F32, kind="ExternalInput").ap()
    wo_d = nc.dram_tensor("woT", [HID, HID], F32, kind="ExternalInput").ap()
    pb_d = nc.dram_tensor("p_basis", [3, NB2], F32, kind="ExternalInput").ap()
    bm_d = nc.dram_tensor("bm2d", [128, LT], F32, kind="ExternalInput").ap()
    out_d = nc.dram_tensor("out", [Q, HID], F32, kind="ExternalOutput").ap()

    with tile.TileContext(nc) as tc:
        pools = []

        def P(name, **kw):
            p = tc.alloc_tile_pool(name=name, bufs=kw.pop("bufs", 1), **kw)
            pools.append(p)
            return p  # NOTE: pools must be released in LIFO order per side

        def rel(*ps):
            for p in ps:
                p.release()
                pools.remove(p)

        cpool = P("cpool")
        bm_sb = cpool.tile([128, LT], F32, name="bm_sb")
        nc.sync.dma_start(bm_sb[:], bm_d)
        p5 = cpool.tile([5, NB2], F32R, name="p5")
        id2 = cpool.tile([2, 2], F32, name="id2")
        make_identity(nc, id2)
        id32 = cpool.tile([32, 32], F32, name="id32")
        make_identity(nc, id32)
        zt = cpool.tile([128, 1], F32, name="zt")
        nc.vector.memset(zt[:], 0.0)

        # ---------------- Phase 2 allocs (early, overlap with phase 1) ---------
        NBA = NB + 2  # 514
        bmP = P("bmP", side="right")
        bmT = bmP.tile([128, JT * NBA], F32, name="bmT")
        gs_all = bmP.tile([128, LT * NBA], F32, name="gs_all")
        nc.sync.dma_start(gs_all.rearrange("p (t c) -> p t c", t=LT),
                          gs_d.rearrange("(t p) c -> p t c", p=128))
        # ---------------- Phase 1: mask matmul + gated keys (kmT) -------------
        kmP = P("kmP")
        kmT = kmP.tile([128, LT * HID], F32, name="kmT")

        ph1 = P("ph1", bufs=1)
        ps1 = P("ps1", space="PSUM")
        p_sb = ph1.tile([3, NB2], F32, name="p_sb")
        nc.sync.dma_start(p_sb[:], pb_d)
        ph3 = ph1.tile([3, NB2], F32R, name="ph3")
        nc.vector.tensor_copy(ph3[:], p_sb[:])
        pl3 = ph1.tile([3, NB2], F32R, name="pl3")
        nc.vector.tensor_tensor(pl3[:], p_sb[:], ph3[:], AL.subtract)
        nc.sync.dma_start(p5[0:1, :], ph3[0:1, :])
        nc.sync.dma_start(p5[1:2, :], pl3[0:1, :])
        nc.sync.dma_start(p5[2:3, :], ph3[1:2, :])
        nc.sync.dma_start(p5[3:4, :], pl3[1:2, :])
        nc.sync.dma_start(p5[4:5, :], ph3[2:3, :])
        k_all = ph1.tile([128, LT * HID], F32R, name="k_all")
        for kc in range(4):
            nc.sync.dma_start(
                k_all[:, kc * 4 * HID:(kc + 1) * 4 * HID]
                .rearrange("p (t h) -> p t h", t=4),
                k_d[kc * 512:(kc + 1) * 512, :]
                .rearrange("(t p) h -> p t h", p=128).bitcast(F32R))
        for mt in range(LT):
            wm_t = ph1.tile([128, L], F32R, name="wm_t", tag="wm", bufs=2)
            nc.sync.dma_start(
                wm_t.rearrange("p (t c) -> p t c", t=LT),
                wm_d[:, mt * 128:(mt + 1) * 128]
                .rearrange("(t p) c -> p t c", p=128).bitcast(F32R))
            mp = ps1.tile([128, HID], F32, name="mp", tag="mp", bufs=2)
            for lt in range(LT):
                for nch in range(2):
                    nc.tensor.matmul(
                        mp[:, nch * 512:(nch + 1) * 512],
                        wm_t[:, lt * 128:(lt + 1) * 128],
                        k_all[:, lt * HID + nch * 512: lt * HID + nch * 512 + 512],
                        start=(lt == 0), stop=(lt == LT - 1))
            sg = ph1.tile([128, HID], F32, name="sg", tag="sg", bufs=2)
            nc.scalar.activation(sg[:], mp[:], AF.Sigmoid, bias=bm_sb[:, mt:mt + 1])
            nc.vector.tensor_tensor(
                kmT[:, mt * HID:(mt + 1) * HID],
                k_all[:, mt * HID:(mt + 1) * HID], sg[:], AL.mult)
        rel(ps1, ph1)

        wfull = P("wfull", side="right")
        ps2 = P("ps2", space="PSUM")
        for jt in range(JT):
            bp = ps2.tile([128, NBA], F32, name="bp", tag="bp", bufs=2)
            for lt in range(LT):
                lhsT = kmT[:, lt * HID + jt * 128: lt * HID + jt * 128 + 128]
                nc.tensor.matmul(bp[:, 0:512], lhsT,
                                 gs_all[:, lt * NBA: lt * NBA + 512],
                                 start=(lt == 0), stop=(lt == LT - 1))
                nc.tensor.matmul(bp[:, 512:514], lhsT,
                                 gs_all[:, lt * NBA + 512: lt * NBA + 514],
                                 start=(lt == 0), stop=(lt == LT - 1))
            nc.vector.tensor_copy(bmT[:, jt * NBA:(jt + 1) * NBA], bp[:])
        rel(ps2, kmP)

        # ---------------- Phase 3: vals, u, Wtilde ---------------------------
        valsP = P("valsP")
        vals_all = valsP.tile([128, 4 * HID], F32, name="vals_all")
        sm = P("sm")
        u_sb = sm.tile([2, HID], F32, name="u_sb")
        ubar = sm.tile([128, JT * 32], F32R, name="ubar")
        wtT = sm.tile([32, HID], F32, name="wtT")
        wt_all = sm.tile([128, JT * 32], F32R, name="wt_all")

        ps3a = P("ps3a", space="PSUM")
        vps = [ps3a.tile([128, HID], F32, name=f"vp{nt}", tag="vp", bufs=4)
               for nt in range(4)]
        for half in range(2):
            wvh = wfull.tile([128, 4 * HID], F32, name="wvh", tag="wf", bufs=2)
            nc.sync.dma_start(
                wvh.rearrange("p (t c) -> p t c", t=4),
                wv_d[half * 512:(half + 1) * 512, :]
                .rearrange("(t p) c -> p t c", p=128))
            for nt in range(4):
                for jt2 in range(4):
                    jt = half * 4 + jt2
                    for nch in range(2):
                        nc.tensor.matmul(
                            vps[nt][:, nch * 512:(nch + 1) * 512],
                            bmT[:, jt * NBA + nt * 128: jt * NBA + nt * 128 + 128],
                            wvh[:, jt2 * HID + nch * 512: jt2 * HID + nch * 512 + 512],
                            start=(jt == 0), stop=(jt == JT - 1))
        for nt in range(4):
            nc.vector.tensor_copy(vals_all[:, nt * HID:(nt + 1) * HID], vps[nt][:])
        rel(ps3a)

        ps3b = P("ps3b", space="PSUM")
        up = ps3b.tile([2, HID], F32, name="up", tag="up")
        for half in range(2):
            wkh = wfull.tile([128, 4 * HID], F32, name="wkh", tag="wf", bufs=2)
            nc.sync.dma_start(
                wkh.rearrange("p (t c) -> p t c", t=4),
                wk_d[half * 512:(half + 1) * 512, :]
                .rearrange("(t p) c -> p t c", p=128))
            for jt2 in range(4):
                jt = half * 4 + jt2
                for nch in range(2):
                    nc.tensor.matmul(
                        up[:, nch * 512:(nch + 1) * 512],
                        bmT[:, jt * NBA + 512: jt * NBA + 514],
                        wkh[:, jt2 * HID + nch * 512: jt2 * HID + nch * 512 + 512],
                        start=(jt == 0), stop=(jt == JT - 1))
        nc.scalar.mul(u_sb[:], up[:], 1.0 / (D ** 0.5))

        nc.vector.tensor_copy(ubar[:], zt[:, 0:1].to_broadcast((128, JT * 32)))
        for c in range(JT):
            tp = ps3b.tile([128, 2], F32, name="tp", tag="tp", bufs=2)
            nc.tensor.transpose(tp[:], u_sb[:, c * 128:(c + 1) * 128], id2[:])
            base = c * 32
            nc.vector.tensor_copy(ubar[0:64, base + 2 * c: base + 2 * c + 1],
                                  tp[0:64, 0:1])
            nc.vector.tensor_copy(ubar[64:128, base + 2 * c + 1: base + 2 * c + 2],
                                  tp[64:128, 0:1])
            nc.vector.tensor_copy(ubar[0:64, base + 16 + 2 * c: base + 16 + 2 * c + 1],
                                  tp[0:64, 1:2])
            nc.vector.tensor_copy(ubar[64:128, base + 17 + 2 * c: base + 18 + 2 * c],
                                  tp[64:128, 1:2])

        wtp = ps3b.tile([32, HID], F32, name="wtp", tag="wtp")
        for half in range(2):
            wqh = wfull.tile([128, 4 * HID], F32R, name="wqh", tag="wf", bufs=2)
            nc.sync.dma_start(
                wqh.rearrange("p (t c) -> p t c", t=4),
                wq_d[half * 512:(half + 1) * 512, :]
                .rearrange("(t p) c -> p t c", p=128).bitcast(F32R))
            for c2 in range(4):
                c = half * 4 + c2
                for nch in range(2):
                    nc.tensor.matmul(
                        wtp[:, nch * 512:(nch + 1) * 512],
                        ubar[:, c * 32:(c + 1) * 32],
                        wqh[:, c2 * HID + nch * 512: c2 * HID + nch * 512 + 512],
                        start=(c == 0), stop=(c == JT - 1))
        nc.scalar.copy(wtT[:], wtp[:])
        for c in range(JT):
            tp2 = ps3b.tile([128, 32], F32, name="tp2", tag="tp2", bufs=2)
            nc.tensor.transpose(tp2[:], wtT[:, c * 128:(c + 1) * 128], id32[:])
            nc.vector.tensor_copy(wt_all[:, c * 32:(c + 1) * 32], tp2[:])
        rel(ps3b, wfull, bmP)

        # ---------------- Phase 4: mu_pre ------------------------------------
        t16P = P("t16P", side="right")
        t16 = t16P.tile([16, 2 * Q], F32, name="t16")
        ph4 = P("ph4")
        ps4 = P("ps4", space="PSUM")
        qt_all = ph4.tile([128, JT * Q], F32R, name="qt_all")
        nc.sync.dma_start(qt_all.rearrange("p (t c) -> p t c", t=JT),
                          qt_d.rearrange("(t p) c -> p t c", p=128).bitcast(F32R))
        mupA = ps4.tile([16, Q], F32, name="mupA", tag="mupA")
        mupB = ps4.tile([16, Q], F32, name="mupB", tag="mupB")
        for kt in range(JT):
            for qc in range(4):
                rhs = qt_all[:, kt * Q + qc * 512: kt * Q + qc * 512 + 512]
                nc.tensor.matmul(mupA[:, qc * 512:(qc + 1) * 512],
                                 wt_all[:, kt * 32: kt * 32 + 16], rhs,
                                 start=(kt == 0), stop=(kt == JT - 1))
                nc.tensor.matmul(mupB[:, qc * 512:(qc + 1) * 512],
                                 wt_all[:, kt * 32 + 16: kt * 32 + 32], rhs,
                                 start=(kt == 0), stop=(kt == JT - 1))
        nc.scalar.copy(t16[:, 0:Q], mupA[:])
        nc.scalar.copy(t16[:, Q:2 * Q], mupB[:])
        rel(ps4, ph4, sm)

        # ---------------- Phase 5: per-(s,h,q) quadratic coefficient grids ----
        woP = P("woP")
        wo = woP.tile([128, JT * HID], F32R, name="wo")
        nc.sync.dma_start(wo.rearrange("p (t c) -> p t c", t=JT),
                          wo_d.rearrange("(t p) c -> p t c", p=128).bitcast(F32R))
        qgP = P("qgP")
        gq1 = [qgP.tile([16, Q], F32R, name=f"gq1_{s}") for s in range(2)]
        gq2 = [qgP.tile([16, Q], F32R, name=f"gq2_{s}") for s in range(2)]
        gq3 = [qgP.tile([16, Q], F32R, name=f"gq3_{s}") for s in range(2)]
        gt = P("gt", side="right")
        gmu = gt.tile([16, Q], F32, name="gmu")
        gsp = gt.tile([16, Q], F32, name="gsp")
        gss = gt.tile([16, Q], F32, name="gss")
        gvs = gt.tile([16, Q], F32, name="gvs", tag="gvs", bufs=1)
        givr = gt.tile([16, Q], F32, name="givr", tag="givr", bufs=1)
        gscr = gt.tile([16, Q], F32, name="gscr", tag="gscr", bufs=1)
        gln = gt.tile([16, Q], F32, name="gln", tag="gln", bufs=1)

        nc.scalar.activation(gmu[:], t16[:, 0:Q], AF.Sigmoid)
        # softplus(x) = ln(exp(x) + 1); input range is ~[-1, 1] so exp is safe
        nc.scalar.activation(gsp[:], t16[:, Q:2 * Q], AF.Exp)
        nc.scalar.activation(gss[:], gsp[:], AF.Ln, bias=1.0)
        nc.vector.tensor_scalar_max(gss[:], gss[:], 1e-6)
        for s in range(2):
            if s > 0:
                gvs = gt.tile([16, Q], F32, name="gvs", tag="gvs", bufs=1)
                givr = gt.tile([16, Q], F32, name="givr", tag="givr", bufs=1)
                gscr = gt.tile([16, Q], F32, name="gscr", tag="gscr", bufs=1)
                gln = gt.tile([16, Q], F32, name="gln", tag="gln", bufs=1)
            nc.vector.tensor_scalar_add(gvs[:], gss[:], SIGMAS[s] ** 2)
            nc.vector.reciprocal_approx_accurate(givr[:], gvs[:], gscr[:])
            nc.scalar.activation(gln[:], gvs[:], AF.Ln, scale=TWO_PI)
            nc.vector.tensor_scalar_mul(gq1[s][:], givr[:], -0.5)
            # q2 = (-2*mu)*q1 = iv*mu ; t3 = (-0.5*mu)*q2 = -0.5*iv*mu^2
            nc.vector.scalar_tensor_tensor(gq2[s][:], gmu[:], -2.0, gq1[s][:],
                                           AL.mult, AL.mult)
            nc.vector.scalar_tensor_tensor(gscr[:], gmu[:], -0.5, gq2[s][:],
                                           AL.mult, AL.mult)
            nc.vector.scalar_tensor_tensor(gq3[s][:], gln[:], -0.5, gscr[:],
                                           AL.mult, AL.add)
        rel(gt, t16P)

        # ---------------- Phase 6: r = exp(g) and context ---------------------
        ctxP = P("ctxP", side="right")
        ctxT = ctxP.tile([128, 8 * Q], F32R, name="ctxT")
        qp = P("qp")
        rp = P("rp")
        tmpP = P("tmpP")
        ps6 = P("ps6", space="PSUM")
        for h in range(H):
            p, odd = divmod(h, 2)
            cxp = ps6.tile([64, Q], F32, name="cxp", tag="cxp", bufs=1)
            for s in range(2):
                qt_t = qp.tile([5, Q], F32R, name="qt_t", tag="qt", bufs=2)
                nc.sync.dma_start(qt_t[0:1, :], gq1[s][h:h + 1, :])
                nc.sync.dma_start(qt_t[1:2, :], gq1[s][h:h + 1, :])
                nc.sync.dma_start(qt_t[2:3, :], gq2[s][h:h + 1, :])
                nc.sync.dma_start(qt_t[3:4, :], gq2[s][h:h + 1, :])
                nc.sync.dma_start(qt_t[4:5, :], gq3[s][h:h + 1, :])
                for t in range(2):
                    nt = 2 * s + t
                    for qh in range(2):
                        gp = ps6.tile([128, 1024], F32, name="gp", tag="gp",
                                      bufs=2)
                        for cc in range(2):
                            nc.tensor.matmul(
                                gp[:, cc * 512:(cc + 1) * 512],
                                p5[:, t * 128:(t + 1) * 128],
                                qt_t[:, qh * 1024 + cc * 512:
                                     qh * 1024 + cc * 512 + 512],
                                start=True, stop=True)
                        rt = rp.tile([128, 1024], F32, name="rt", tag="rt",
                                     bufs=3)
                        nc.scalar.activation(rt[:], gp[:], AF.Exp)
                        for cc in range(2):
                            qc = qh * 2 + cc
                            nc.tensor.matmul(
                                cxp[:, qc * 512:(qc + 1) * 512],
                                vals_all[:, nt * HID + h * D:
                                         nt * HID + h * D + D],
                                rt[:, cc * 512:(cc + 1) * 512],
                                start=(s == 0 and t == 0),
                                stop=(s == 1 and t == 1),
                                skip_group_check=True)
            if not odd:
                nc.vector.tensor_copy(ctxT[0:64, p * Q:(p + 1) * Q], cxp[:])
            else:
                t64 = tmpP.tile([64, Q], F32R, name="t64", tag="t64", bufs=2)
                nc.vector.tensor_copy(t64[:], cxp[:])
                nc.sync.dma_start(ctxT[64:128, p * Q:(p + 1) * Q], t64[:])
        rel(ps6, tmpP, rp, qp, qgP)

        # ---------------- Phase 7: output projection --------------------------
        outP = P("outP")
        ps7 = P("ps7", space="PSUM")
        for qi in range(QTI):
            op = ps7.tile([128, HID], F32, name="op", tag="op", bufs=2)
            for jt in range(JT):
                for och in range(2):
                    nc.tensor.matmul(
                        op[:, och * 512:(och + 1) * 512],
                        ctxT[:, jt * Q + qi * 128: jt * Q + qi * 128 + 128],
                        wo[:, jt * HID + och * 512: jt * HID + och * 512 + 512],
                        start=(jt == 0), stop=(jt == JT - 1))
            ob = outP.tile([128, HID], F32, name="ob", tag="ob", bufs=2)
            nc.vector.tensor_copy(ob[:], op[:])
            nc.sync.dma_start(out_d[qi * 128:(qi + 1) * 128, :], ob[:])
        rel(ps7, outP, ctxP, woP, valsP, cpool)

    nc.compile()
    return nc


def _host_prep(W_mask, Wq, Wk, Wv, Wo, w_mu, w_sigma, Gs, b_mask):
    Gs = np.asarray(Gs, np.float32)
    perm = np.concatenate([np.arange(0, NB, 2), np.arange(1, NB, 2)])
    gs_aug = np.concatenate(
        [Gs[:, perm],
         (Gs @ np.asarray(w_mu, np.float32))[:, None],
         (Gs @ np.asarray(w_sigma, np.float32))[:, None]], axis=1)
    gs_aug = np.ascontiguousarray(gs_aug, np.float32)
    lin = np.linspace(0.0, 1.0, NB2, dtype=np.float64)
    p_basis = np.stack([lin * lin, lin, np.ones_like(lin)]).astype(np.float32)
    bm2d = np.ascontiguousarray(
        np.asarray(b_mask, np.float32).reshape(LT, 128).T)
    return {
        "wmT": np.ascontiguousarray(np.asarray(W_mask, np.float32).T),
        "gs_aug": gs_aug,
        "wvT": np.ascontiguousarray(np.asarray(Wv, np.float32).T),
        "wkT": np.ascontiguousarray(np.asarray(Wk, np.float32).T),
        "wq": np.ascontiguousarray(np.asarray(Wq, np.float32)),
        "woT": np.ascontiguousarray(np.asarray(Wo, np.float32).T),
        "p_basis": p_basis,
        "bm2d": bm2d,
    }


_NC_CACHE = {}


def _get_nc():
    if "nc" not in _NC_CACHE:
        _NC_CACHE["nc"] = build_nc()
    return _NC_CACHE["nc"]


def kernel(k, query, W_mask, b_mask, Wq, Wk, Wv, Wo, w_mu, w_sigma,
           Gs, basis_mu, basis_sigma, _trace=False):
    k = np.asarray(k, np.float32)
    query = np.asarray(query, np.float32)
    shared = _host_prep(W_mask, Wq, Wk, Wv, Wo, w_mu, w_sigma, Gs, b_mask)
    in_maps = []
    for b in range(B):
        m = dict(shared)
        m["k"] = np.ascontiguousarray(k[b])
        m["qt"] = np.ascontiguousarray(
            query[b].transpose(0, 2, 1).reshape(HID, Q))
        in_maps.append(m)
    nc = _get_nc()
    tkw = {"tmpdir": "/tmp/bass_ntff"} if _trace else {}
    if _trace:
        import os as _os
        import shutil as _sh
        _sh.rmtree("/tmp/bass_ntff", ignore_errors=True)
        _os.makedirs("/tmp/bass_ntff", exist_ok=True)
    res = run_bass_kernel_spmd(nc, in_maps, core_ids=list(range(B)),
                               trace=_trace, **tkw)
    out = np.stack([res.results[b]["out"] for b in range(B)])
    if _trace:
        return out, res
    return out



# revision 33
# speedup vs baseline: 1.2237x; 1.2237x over previous
"""Trainium2 Bass kernel for nn_LongTermAttention (continuous-basis long-term attention).

Strategy: pure data-parallel over batch (B=8 -> one batch element per NeuronCore).
Per core, restructured from the 905us baseline for continuous PE occupancy:

  - all big matmul operands live in SBUF as bf16 (halves DMA + SBUF, full-rate PE)
  - every weight / query tile is prefetched during phase 1 (mask matmul)
  - v_mu/v_sigma extracted from Bmat by DVE tensor_tensor_reduce (kills the
    128 2-wide matmuls + their LDWEIGHTS)
  - mu/sigma score rows computed as one 32-row matmul chain; the sigmoid/
    softplus/var grid math runs stacked [32, Q] overlapping the vals matmuls
  - per-(head, sigma) Gaussian-quadratic coefficient rows packed in one
    [80, Q] tile per sigma (rows 5h..5h+5) built by 10 partition-strided DMAs;
    stationary quintic basis replicated to [80, 256] once
  - phase 6/7 fused into a q-block pipeline: per 512 query columns, all heads'
    g-matmul -> Exp -> ctx-matmul run with the output projection of the
    previous q-block filling PE while ACT drains
"""
import os
from contextlib import ExitStack

import numpy as np
import ml_dtypes

import concourse.bass as bass
import concourse.tile as tile
from concourse import bacc, mybir
from concourse.bass_utils import run_bass_kernel_spmd
from concourse.masks import make_identity

F32 = mybir.dt.float32
F32R = mybir.dt.float32r
BF16 = mybir.dt.bfloat16
AF = mybir.ActivationFunctionType
AL = mybir.AluOpType

L = 2048          # memory length
NB = 512          # num basis
NB2 = 256         # per-sigma basis count
HID = 1024
H = 16
D = 64
B = 8
Q = 2048
LT = L // 128     # 16
JT = HID // 128   # 8
QTI = Q // 128    # 16
SIGMAS = (0.005, 0.01)
TWO_PI = 6.283185307179586


def build_nc():
    nc = bacc.Bacc("TRN2", target_bir_lowering=False, debug=False)

    k_d = nc.dram_tensor("k", [L, HID], BF16, kind="ExternalInput").ap()
    qt_d = nc.dram_tensor("qt", [HID, Q], BF16, kind="ExternalInput").ap()
    wm_d = nc.dram_tensor("wmT", [L, L], BF16, kind="ExternalInput").ap()
    gsh_d = nc.dram_tensor("gsh", [L, NB], BF16, kind="ExternalInput").ap()
    gsl_d = nc.dram_tensor("gsl", [L, NB], BF16, kind="ExternalInput").ap()
    wv_d = nc.dram_tensor("wvT", [HID, HID], BF16, kind="ExternalInput").ap()
    wk_d = nc.dram_tensor("wkT", [HID, HID], BF16, kind="ExternalInput").ap()
    wq_d = nc.dram_tensor("wq", [HID, HID], BF16, kind="ExternalInput").ap()
    wo_d = nc.dram_tensor("woT", [HID, HID], BF16, kind="ExternalInput").ap()
    pb_d = nc.dram_tensor("p_basis", [3, NB2], F32, kind="ExternalInput").ap()
    w2_d = nc.dram_tensor("w2", [128, 2 * NB], BF16, kind="ExternalInput").ap()
    bm_d = nc.dram_tensor("bm2d", [128, LT], F32, kind="ExternalInput").ap()
    sq_d = nc.dram_tensor("sigsq", [64, 1], F32, kind="ExternalInput").ap()
    psel_d = nc.dram_tensor("psel", [48, 3 * 80], F32, kind="ExternalInput").ap()
    out_d = nc.dram_tensor("out", [Q, HID], F32, kind="ExternalOutput").ap()

    with tile.TileContext(nc) as tc:
        pools = []

        def P(name, **kw):
            p = tc.alloc_tile_pool(name=name, bufs=kw.pop("bufs", 1), **kw)
            pools.append(p)
            return p  # NOTE: pools must be released in LIFO order per side

        def rel(*ps):
            for p in ps:
                p.release()
                pools.remove(p)

        # ---------------- constants (whole-kernel lifetime) -------------------
        cpool = P("cpool")
        bm_sb = cpool.tile([128, LT], F32, name="bm_sb")
        nc.sync.dma_start(bm_sb[:], bm_d)
        sq_sb = cpool.tile([64, 1], F32, name="sq_sb")
        nc.sync.dma_start(sq_sb[:], sq_d)
        psel = cpool.tile([48, 3 * 80], F32R, name="psel")
        nc.sync.dma_start(psel[:], psel_d.bitcast(F32R))
        w2_sb = cpool.tile([128, 2 * NB], BF16, name="w2_sb")
        nc.sync.dma_start(w2_sb[:], w2_d)
        id2 = cpool.tile([2, 2], F32, name="id2")
        make_identity(nc, id2)
        id32 = cpool.tile([32, 32], F32, name="id32")
        make_identity(nc, id32)
        p5 = cpool.tile([5, NB2], F32R, name="p5")
        p_sb = cpool.tile([3, NB2], F32, name="p_sb")
        nc.sync.dma_start(p_sb[:], pb_d)
        ph3 = cpool.tile([3, NB2], F32R, name="ph3")
        nc.vector.tensor_copy(ph3[:], p_sb[:])
        pl3 = cpool.tile([3, NB2], F32R, name="pl3")
        nc.vector.tensor_tensor(pl3[:], p_sb[:], ph3[:], AL.subtract)
        # p5 rows = [lin2_hi, lin2_lo, lin_hi, lin_lo, 1]
        nc.sync.dma_start(p5[0:1, :], ph3[0:1, :])
        nc.sync.dma_start(p5[1:2, :], pl3[0:1, :])
        nc.sync.dma_start(p5[2:3, :], ph3[1:2, :])
        nc.sync.dma_start(p5[3:4, :], pl3[1:2, :])
        nc.sync.dma_start(p5[4:5, :], ph3[2:3, :])

        # ---------------- long-lived phase-2/3/6 tiles (right side) -----------
        rightP = P("rightP", side="right")
        bmT_hi = rightP.tile([128, JT * NB], BF16, name="bmT_hi")
        bmT_lo = rightP.tile([128, JT * NB], BF16, name="bmT_lo")
        vals_all = rightP.tile([128, 4 * HID], F32, name="vals_all")

        # weights/query prefetch pool (lives until vals); below kmP on stack
        wfull = P("wfull")
        wv_sb = wfull.tile([128, JT * HID], BF16, name="wv_sb")
        wk_sb = wfull.tile([128, JT * HID], BF16, name="wk_sb")
        wq_sb = wfull.tile([128, JT * HID], BF16, name="wq_sb")

        gsS = P("gsS")
        kmP = P("kmP")
        kmT = kmP.tile([128, LT * HID], BF16, name="kmT")

        # ---------------- Phase 1: mask matmul + gated keys (kmT) -------------
        ph1 = P("ph1", bufs=1)
        ps1 = P("ps1", space="PSUM")
        k_all = ph1.tile([128, LT * HID], BF16, name="k_all")
        for kc in range(4):
            nc.sync.dma_start(
                k_all[:, kc * 4 * HID:(kc + 1) * 4 * HID]
                .rearrange("p (t h) -> p t h", t=4),
                k_d[kc * 512:(kc + 1) * 512, :]
                .rearrange("(t p) h -> p t h", p=128))
        wm_tiles = []
        for mt in range(2):
            wm_t = ph1.tile([128, L], BF16, name="wm_t", tag="wm", bufs=2)
            nc.sync.dma_start(
                wm_t.rearrange("p (t c) -> p t c", t=LT),
                wm_d[:, mt * 128:(mt + 1) * 128]
                .rearrange("(t p) c -> p t c", p=128))
            wm_tiles.append(wm_t)
        # background prefetch (consumed from phase 2 on)
        for w_sb, w_d in ((wk_sb, wk_d), (wq_sb, wq_d), (wv_sb, wv_d)):
            for half in range(2):
                nc.sync.dma_start(
                    w_sb[:, half * 4 * HID:(half + 1) * 4 * HID]
                    .rearrange("p (t c) -> p t c", t=4),
                    w_d[half * 512:(half + 1) * 512, :]
                    .rearrange("(t p) c -> p t c", p=128))

        for mt in range(LT):
            if mt < 2:
                wm_t = wm_tiles[mt]
            else:
                wm_t = ph1.tile([128, L], BF16, name="wm_t", tag="wm", bufs=2)
                nc.sync.dma_start(
                    wm_t.rearrange("p (t c) -> p t c", t=LT),
                    wm_d[:, mt * 128:(mt + 1) * 128]
                    .rearrange("(t p) c -> p t c", p=128))
            mp = ps1.tile([128, HID], F32, name="mp", tag="mp", bufs=2)
            for lt in range(LT):
                for nch in range(2):
                    nc.tensor.matmul(
                        mp[:, nch * 512:(nch + 1) * 512],
                        wm_t[:, lt * 128:(lt + 1) * 128],
                        k_all[:, lt * HID + nch * 512: lt * HID + nch * 512 + 512],
                        start=(lt == 0), stop=(lt == LT - 1))
            sg = ph1.tile([128, HID], BF16, name="sg", tag="sg", bufs=2)
            nc.scalar.activation(sg[:], mp[:], AF.Sigmoid, bias=bm_sb[:, mt:mt + 1])
            nc.vector.tensor_tensor(
                kmT[:, mt * HID:(mt + 1) * HID],
                k_all[:, mt * HID:(mt + 1) * HID], sg[:], AL.mult)
        rel(ps1, ph1)

        # ---------------- Phase 2: Bmat (+ v extraction on DVE) ---------------
        smP = P("smP", side="right")
        v_f32 = smP.tile([128, 2 * JT], F32, name="v_f32")
        v_bf = smP.tile([128, 2 * JT], BF16, name="v_bf")
        u_sb = smP.tile([2, HID], F32, name="u_sb")
        ubar = smP.tile([128, JT * 32], BF16, name="ubar")
        wtT = smP.tile([32, HID], F32, name="wtT")
        # 64-wide blocks: mu rows -> cols 0:16, sigma rows -> cols 32:48 so the
        # mu_pre PSUM output lands at partition bases 0 / 32 (HW alignment)
        wt_all = smP.tile([128, JT * 64], BF16, name="wt_all")

        ps2 = P("ps2", space="PSUM")
        up = ps2.tile([2, HID], F32, name="up")
        for half in range(2):
            bps = [ps2.tile([128, NB], F32, name=f"bp{half}_{j}", tag=f"bp{j}",
                            bufs=1) for j in range(4)]
            for lt in range(LT):
                ghi = gsS.tile([128, NB], BF16, name="ghi", tag="ghi", bufs=3)
                nc.sync.dma_start(ghi[:], gsh_d[lt * 128:(lt + 1) * 128, :])
                glo = gsS.tile([128, NB], BF16, name="glo", tag="glo", bufs=3)
                nc.sync.dma_start(glo[:], gsl_d[lt * 128:(lt + 1) * 128, :])
                for j4 in range(4):
                    jt = half * 4 + j4
                    lhsT = kmT[:, lt * HID + jt * 128: lt * HID + jt * 128 + 128]
                    nc.tensor.matmul(bps[j4][:], lhsT, ghi[:],
                                     start=(lt == 0), stop=False)
                    nc.tensor.matmul(bps[j4][:], lhsT, glo[:],
                                     start=False, stop=(lt == LT - 1))
            for j4 in range(4):
                jt = half * 4 + j4
                # split Bmat into bf16 hi + lo for later exact-ish matmuls
                nc.vector.tensor_copy(bmT_hi[:, jt * NB:(jt + 1) * NB],
                                      bps[j4][:])
                nc.vector.tensor_tensor(bmT_lo[:, jt * NB:(jt + 1) * NB],
                                        bps[j4][:],
                                        bmT_hi[:, jt * NB:(jt + 1) * NB],
                                        AL.subtract)
                # v[:, jt*2+c] = sum_n w2[c, n] * Bmat[n, hid-block jt]
                # (tensor_tensor_reduce faults the HW; use DVE mult + ACT
                #  Identity with accum_out for the free-dim reduction)
                for c in range(2):
                    scr = smP.tile([128, NB], BF16, name="scr", tag="scr",
                                   bufs=2)
                    nc.vector.tensor_tensor(
                        scr[:], bmT_hi[:, jt * NB:(jt + 1) * NB],
                        w2_sb[:, c * NB:(c + 1) * NB], AL.mult)
                    scr2 = smP.tile([128, NB], BF16, name="scr2", tag="scr2",
                                    bufs=2)
                    nc.scalar.activation(
                        scr2[:], scr[:], AF.Identity,
                        accum_out=v_f32[:, jt * 2 + c: jt * 2 + c + 1])
        nc.vector.tensor_copy(v_bf[:], v_f32[:])
        # u[2, HID] = sum_jt v_jt.T @ WkT[jt-block]   (1/sqrt(D) folded in w2)
        for jt in range(JT):
            for nch in range(2):
                nc.tensor.matmul(
                    up[:, nch * 512:(nch + 1) * 512],
                    v_bf[:, jt * 2: jt * 2 + 2],
                    wk_sb[:, jt * HID + nch * 512: jt * HID + nch * 512 + 512],
                    start=(jt == 0), stop=(jt == JT - 1))
        nc.scalar.copy(u_sb[:], up[:])
        rel(ps2)
        rel(kmP)

        # ---------------- Phase 3: ubar, Wtilde, mu_pre -----------------------
        ps3a = P("ps3a", space="PSUM")
        nc.vector.memset(ubar[:], 0.0)
        for c in range(JT):
            tp = ps3a.tile([128, 2], F32, name="tp", tag="tp", bufs=2)
            nc.tensor.transpose(tp[:], u_sb[:, c * 128:(c + 1) * 128], id2[:])
            base = c * 32
            nc.vector.tensor_copy(ubar[0:64, base + 2 * c: base + 2 * c + 1],
                                  tp[0:64, 0:1])
            nc.vector.tensor_copy(ubar[64:128, base + 2 * c + 1: base + 2 * c + 2],
                                  tp[64:128, 0:1])
            nc.vector.tensor_copy(ubar[0:64, base + 16 + 2 * c: base + 16 + 2 * c + 1],
                                  tp[0:64, 1:2])
            nc.vector.tensor_copy(ubar[64:128, base + 17 + 2 * c: base + 18 + 2 * c],
                                  tp[64:128, 1:2])
        rel(ps3a)

        ps3b = P("ps3b", space="PSUM")
        wtp = ps3b.tile([32, HID], F32, name="wtp")
        for c in range(JT):
            for nch in range(2):
                nc.tensor.matmul(
                    wtp[:, nch * 512:(nch + 1) * 512],
                    ubar[:, c * 32:(c + 1) * 32],
                    wq_sb[:, c * HID + nch * 512: c * HID + nch * 512 + 512],
                    start=(c == 0), stop=(c == JT - 1))
        nc.scalar.copy(wtT[:], wtp[:])
        nc.vector.memset(wt_all[:], 0.0)
        for c in range(JT):
            tp2 = ps3b.tile([128, 32], F32, name="tp2", tag="tp2", bufs=2)
            nc.tensor.transpose(tp2[:], wtT[:, c * 128:(c + 1) * 128], id32[:])
            nc.vector.tensor_copy(wt_all[:, c * 64: c * 64 + 16], tp2[:, 0:16])
            nc.vector.tensor_copy(wt_all[:, c * 64 + 32: c * 64 + 48],
                                  tp2[:, 16:32])
        rel(ps3b)

        # grid + phase-6 tiles (right side, allocated after phase-1 SBUF freed)
        gt = P("gt", side="right")
        qtl = [gt.tile([80, Q], F32R, name=f"qtl{s}") for s in range(2)]
        gmu = gt.tile([64, Q], F32, name="gmu")
        gsp = gt.tile([64, Q], F32, name="gsp")
        gvs = gt.tile([64, Q], F32, name="gvs")
        givr = gt.tile([64, Q], F32, name="givr")
        # f32r outputs: the psel matmul consumes these, and the BIR verifier
        # requires f32r matmul inputs to be produced rounded-to-f32r
        gq1 = gt.tile([64, Q], F32R, name="gq1")
        gq2 = gt.tile([64, Q], F32R, name="gq2")
        gq3 = gt.tile([64, Q], F32R, name="gq3")
        gscr, gln, gt3 = gsp, gsp, gvs  # dead-tile reuse (exp dead after Ln)

        # mu_pre: rows 0:16 mu-channel, 32:48 sigma-channel, per qc chunk
        ps3c = P("ps3c", space="PSUM")
        mup = [ps3c.tile([64, 512], F32, name=f"mup{qc}") for qc in range(4)]
        for kt in range(JT):
            qts = gsS.tile([128, Q], BF16, name="qts", tag="qts", bufs=2)
            nc.sync.dma_start(qts[:], qt_d[kt * 128:(kt + 1) * 128, :])
            for qc in range(4):
                nc.tensor.matmul(
                    mup[qc][:], wt_all[:, kt * 64:(kt + 1) * 64],
                    qts[:, qc * 512:(qc + 1) * 512],
                    start=(kt == 0), stop=(kt == JT - 1))

        # ---------------- Phase 4: grids (stacked [64, Q]: s rows at 32s) -----
        nc.vector.memset(gmu[:], 0.0)   # rows 16:32 / 48:64 are padding but
        nc.vector.memset(gsp[:], 0.0)   # full-tile ops below read all 64 rows
        for qc in range(4):
            for s in range(2):
                nc.scalar.activation(gmu[32 * s:32 * s + 16, qc * 512:(qc + 1) * 512],
                                     mup[qc][0:16, :], AF.Sigmoid)
                nc.scalar.activation(gsp[32 * s:32 * s + 16, qc * 512:(qc + 1) * 512],
                                     mup[qc][32:48, :], AF.Exp)
        # var = max(softplus, 1e-6) + sigma_s^2 ; softplus = ln(exp(x) + 1)
        nc.scalar.activation(gvs[:], gsp[:], AF.Ln, bias=1.0)
        nc.vector.tensor_scalar(gvs[:], gvs[:], 1e-6, sq_sb[:, 0:1],
                                AL.max, AL.add)
        nc.vector.reciprocal_approx_accurate(givr[:], gvs[:], gscr[:])
        nc.scalar.activation(gln[:], gvs[:], AF.Ln, scale=TWO_PI)  # gln == gsp
        nc.vector.tensor_scalar_mul(gq1[:], givr[:], -0.5)         # in-place
        nc.vector.scalar_tensor_tensor(gq2[:], gmu[:], -2.0, gq1[:],
                                       AL.mult, AL.mult)           # gq2 == gscr
        nc.vector.scalar_tensor_tensor(gt3[:], gmu[:], -0.5, gq2[:],
                                       AL.mult, AL.mult)           # gt3 == gvs
        nc.vector.scalar_tensor_tensor(gq3[:], gln[:], -0.5, gt3[:],
                                       AL.mult, AL.add)            # gq3 == gmu

        # pack per-(s, h) coefficient rows into qtl[s] rows 5h+r via selection
        # matmuls: row 5h+r <- [gq1, gq1, gq2, gq2, gq3][r] (head h, sigma s)
        psQ = P("psQ", space="PSUM")
        gqs = [gq1, gq2, gq3]
        for s in range(2):
            for ch in range(4):
                # PE faults on matmul outputs with 80 partitions; split 64+16
                qps = psQ.tile([64, 512], F32, name="qps", tag="qps", bufs=1)
                qp2 = psQ.tile([16, 512], F32, name="qp2", tag="qp2", bufs=1)
                for r3 in range(3):
                    lhs = psel[32 * s:32 * s + 16, r3 * 80:(r3 + 1) * 80]
                    rhs = gqs[r3][32 * s:32 * s + 16, ch * 512:(ch + 1) * 512]
                    nc.tensor.matmul(qps[:], lhs[:, 0:64], rhs,
                                     start=(r3 == 0), stop=(r3 == 2))
                    nc.tensor.matmul(qp2[:], lhs[:, 64:80], rhs,
                                     start=(r3 == 0), stop=(r3 == 2))
                nc.vector.tensor_copy(qtl[s][0:64, ch * 512:(ch + 1) * 512],
                                      qps[:])
                nc.vector.tensor_copy(qtl[s][64:80, ch * 512:(ch + 1) * 512],
                                      qp2[:])

        # ---------------- Phase 3e: vals (overlaps grid math on PE) -----------
        ps3d = P("ps3d", space="PSUM")
        for nt in range(4):
            vps = ps3d.tile([128, HID], F32, name=f"vp{nt}", tag="vp", bufs=1)
            for jt in range(JT):
                for nch in range(2):
                    rhs = wv_sb[:, jt * HID + nch * 512: jt * HID + nch * 512 + 512]
                    sl = slice(jt * NB + nt * 128, jt * NB + nt * 128 + 128)
                    nc.tensor.matmul(vps[:, nch * 512:(nch + 1) * 512],
                                     bmT_hi[:, sl], rhs,
                                     start=(jt == 0), stop=False)
                    nc.tensor.matmul(vps[:, nch * 512:(nch + 1) * 512],
                                     bmT_lo[:, sl], rhs,
                                     start=False, stop=(jt == JT - 1))
            nc.vector.tensor_copy(
                vals_all[:, nt * HID:(nt + 1) * HID], vps[:])
        rel(ps3d, psQ, ps3c)
        rel(gsS)
        rel(wfull)

        # ---------------- Phase 6/7 fused: q-block pipeline -------------------
        qp = P("qp")
        wo_sb = qp.tile([128, JT * HID], BF16, name="wo_sb")
        for half in range(2):
            nc.sync.dma_start(
                wo_sb[:, half * 4 * HID:(half + 1) * 4 * HID]
                .rearrange("p (t c) -> p t c", t=4),
                wo_d[half * 512:(half + 1) * 512, :]
                .rearrange("(t p) c -> p t c", p=128))
        rp = P("rp")
        obP = P("obP")
        ps6 = P("ps6", space="PSUM")
        ps7 = P("ps7", space="PSUM")
        for qb in range(4):
            q0 = qb * 512
            ctxq = qp.tile([128, JT * 512], BF16, name="ctxq", tag="ctxq", bufs=2)
            for pair in range(8):
                cxps = []
                for hh in range(2):
                    h = 2 * pair + hh
                    cxp = ps6.tile([64, 512], F32, name="cxp", tag="cxp", bufs=2)
                    cxps.append(cxp)
                    stage = rp.tile([5, 1024], F32R, name="stage", tag="st",
                                    bufs=4)
                    for s in range(2):
                        nc.sync.dma_start(stage[:, s * 512:(s + 1) * 512],
                                          qtl[s][5 * h:5 * h + 5, q0:q0 + 512])
                    for s in range(2):
                        gp = ps6.tile([128, 1024], F32, name="gp", tag="gp",
                                      bufs=2)
                        for t in range(2):
                            nc.tensor.matmul(
                                gp[:, t * 512:(t + 1) * 512],
                                p5[:, t * 128:(t + 1) * 128],
                                stage[:, s * 512:(s + 1) * 512],
                                start=True, stop=True)
                        rt = rp.tile([128, 1024], F32, name="rt", tag="rt",
                                     bufs=3)
                        nc.scalar.activation(rt[:], gp[:], AF.Exp)
                        for t in range(2):
                            nt = 2 * s + t
                            nc.tensor.matmul(
                                cxp[:],
                                vals_all[:, nt * HID + h * D: nt * HID + h * D + D],
                                rt[:, t * 512:(t + 1) * 512],
                                start=(s == 0 and t == 0),
                                stop=(s == 1 and t == 1),
                                skip_group_check=True)
                nc.vector.tensor_copy(ctxq[0:64, pair * 512:(pair + 1) * 512],
                                      cxps[0][:])
                t64 = rp.tile([64, 512], BF16, name="t64", tag="t64", bufs=2)
                nc.vector.tensor_copy(t64[:], cxps[1][:])
                nc.sync.dma_start(ctxq[64:128, pair * 512:(pair + 1) * 512],
                                  t64[:])
            # output projection for this q-block
            for qi in range(4):
                for och in range(2):
                    op = ps7.tile([128, 512], F32, name="op", tag="op", bufs=2)
                    for jt in range(JT):
                        nc.tensor.matmul(
                            op[:],
                            ctxq[:, jt * 512 + qi * 128: jt * 512 + qi * 128 + 128],
                            wo_sb[:, jt * HID + och * 512: jt * HID + och * 512 + 512],
                            start=(jt == 0), stop=(jt == JT - 1))
                    ob = obP.tile([128, 512], F32, name="ob", tag="ob", bufs=3)
                    nc.vector.tensor_copy(ob[:], op[:])
                    nc.sync.dma_start(
                        out_d[q0 + qi * 128: q0 + qi * 128 + 128,
                              och * 512:(och + 1) * 512], ob[:])
        rel(ps7, ps6, obP, rp, qp, gt, smP, rightP, cpool)

    nc.compile()
    return nc


def _host_prep(W_mask, Wq, Wk, Wv, Wo, w_mu, w_sigma, Gs, b_mask):
    bf = ml_dtypes.bfloat16
    Gs = np.asarray(Gs, np.float32)
    perm = np.concatenate([np.arange(0, NB, 2), np.arange(1, NB, 2)])
    w2 = np.stack([np.asarray(w_mu, np.float32)[perm],
                   np.asarray(w_sigma, np.float32)[perm]]) / (D ** 0.5)
    w2 = np.tile(w2.reshape(1, 2 * NB), (128, 1))  # replicated across partitions
    gsP = np.ascontiguousarray(Gs[:, perm])
    gsh = gsP.astype(bf)
    gsl = (gsP - gsh.astype(np.float32)).astype(bf)
    lin = np.linspace(0.0, 1.0, NB2, dtype=np.float64)
    p_basis = np.stack([lin * lin, lin, np.ones_like(lin)]).astype(np.float32)
    bm2d = np.ascontiguousarray(
        np.asarray(b_mask, np.float32).reshape(LT, 128).T)
    sigsq = np.zeros((64, 1), np.float32)
    sigsq[0:16] = SIGMAS[0] ** 2
    sigsq[32:48] = SIGMAS[1] ** 2
    psel = np.zeros((48, 3 * 80), np.float32)
    for h in range(H):
        for r0 in (0, 32):
            psel[r0 + h, 0 * 80 + 5 * h + 0] = 1.0
            psel[r0 + h, 0 * 80 + 5 * h + 1] = 1.0
            psel[r0 + h, 1 * 80 + 5 * h + 2] = 1.0
            psel[r0 + h, 1 * 80 + 5 * h + 3] = 1.0
            psel[r0 + h, 2 * 80 + 5 * h + 4] = 1.0
    return {
        "wmT": np.ascontiguousarray(np.asarray(W_mask, np.float32).T).astype(bf),
        "gsh": gsh, "gsl": gsl,
        "wvT": np.ascontiguousarray(np.asarray(Wv, np.float32).T).astype(bf),
        "wkT": np.ascontiguousarray(np.asarray(Wk, np.float32).T).astype(bf),
        "wq": np.ascontiguousarray(np.asarray(Wq, np.float32)).astype(bf),
        "woT": np.ascontiguousarray(np.asarray(Wo, np.float32).T).astype(bf),
        "p_basis": p_basis,
        "w2": np.ascontiguousarray(w2).astype(bf),
        "bm2d": bm2d,
        "sigsq": sigsq,
        "psel": psel,
    }


_NC_CACHE = {}


def _get_nc():
    if "nc" not in _NC_CACHE:
        _NC_CACHE["nc"] = build_nc()
    return _NC_CACHE["nc"]


def kernel(k, query, W_mask, b_mask, Wq, Wk, Wv, Wo, w_mu, w_sigma,
           Gs, basis_mu, basis_sigma, _trace=False):
    bf = ml_dtypes.bfloat16
    k = np.asarray(k, np.float32)
    query = np.asarray(query, np.float32)
    shared = _host_prep(W_mask, Wq, Wk, Wv, Wo, w_mu, w_sigma, Gs, b_mask)
    in_maps = []
    for b in range(B):
        m = dict(shared)
        m["k"] = np.ascontiguousarray(k[b]).astype(bf)
        m["qt"] = np.ascontiguousarray(
            query[b].transpose(0, 2, 1).reshape(HID, Q)).astype(bf)
        in_maps.append(m)
    nc = _get_nc()
    tkw = {"tmpdir": "/tmp/bass_ntff"} if _trace else {}
    if _trace:
        import os as _os
        import shutil as _sh
        _sh.rmtree("/tmp/bass_ntff", ignore_errors=True)
        _os.makedirs("/tmp/bass_ntff", exist_ok=True)
    res = run_bass_kernel_spmd(nc, in_maps, core_ids=list(range(B)),
                               trace=_trace, **tkw)
    out = np.stack([res.results[b]["out"] for b in range(B)])
    if _trace:
        return out, res
    return out
